# revision 1
# baseline (speedup 1.0000x reference)
"""Trainium2 Bass kernel for nn_ClusteringLayer: per-cacheline serial
near-duplicate clustering (threshold 0.1, cacheline 64).

Algorithm (per line of 64 values, walked left to right): each value snaps
to the FIRST earlier base within |d| < 0.1, else becomes a new base.

Mapping: lines live on partitions (L lines per partition per tile). For
step t, slots 0..t of each line are compared against value t. A packed key
kb_j = j + 1.5 + x_j/16 carries (slot, value) through a penalized
min-reduce, so the first matching base's index AND value come out of one
reduce with no gather. Non-base slots are "punched" by adding 1e30 to
their key. Match decisions are exact: d computed in fp32, Square(d*2^20)
= d^2*2^40 exactly (pow2 scale commutes with rounding), compared against
V1*2^40 where V1 = nextafter(round(0.1f^2), 0); penalized-vs-match
separation is >= 1024 vs keys < 67. Only the output value passes through
the key packing (max abs err ~6e-5, zero decision error).

Engine split per step: GPSIMD subtract, ACT square (both read only the
pristine x tile, so they run arbitrarily ahead), DVE does
(sq - V1S) max kb -> segmented min-reduce; ACT also computes the punch
mask from the reduced key, DVE applies the punch.

Sharding: pure data parallel over lines, 100352 lines per core x 8 cores.
"""
import numpy as np

import concourse.bacc as bacc
import concourse.mybir as mybir
import concourse.tile as tile
from concourse.bass_utils import run_bass_kernel_spmd

F32 = mybir.dt.float32
OP = mybir.AluOpType
AF = mybir.ActivationFunctionType

SHAPE = (64, 64, 112, 112)
C = 64                      # cacheline
NCORES = 8
TOTAL = 64 * 64 * 112 * 112     # 51380224
NLINES = TOTAL // C             # 802816
LPC = NLINES // NCORES          # 100352 lines per core
LPP = LPC // 128                # 784 lines per partition
L = 49                          # lines per partition per tile
NT = LPP // L                   # tiles per core
BUFS = 2
DVE_SUB_T = 64                  # steps t >= this run the subtract on DVE
SPLIT_STT = False               # split z-combine into early bulk + punch tail

# exact threshold: match <=> d^2 <= V1 <=> (d*2^20)^2 <= V1*2^40
V2 = np.float32(np.float32(0.1) * np.float32(0.1))
V1 = np.nextafter(V2, np.float32(0), dtype=np.float32)
SQ_SCALE = float(np.float32(2.0 ** 20))
V1S = float(np.float32(V1.astype(np.float64) * 2.0 ** 40))
CM_SCALE = -1e31

_CACHE = {}


def _build():
    nc = bacc.Bacc("TRN2", target_bir_lowering=False, debug=False)
    x_d = nc.dram_tensor("x", [128, LPP * C], F32, kind="ExternalInput")
    o_d = nc.dram_tensor("out", [128, LPP * C], F32, kind="ExternalOutput")

    with tile.TileContext(nc) as tc:
        with (
            tc.tile_pool(name="const", bufs=1) as cpool,
            tc.tile_pool(name="work", bufs=BUFS) as pool,
            tc.tile_pool(name="chain", bufs=1) as chpool,
        ):
            iotah = cpool.tile([128, C], F32)       # j + 1.5
            sqsc_t = cpool.tile([128, 1], F32)      # 2^20 (Square input scale)
            nv1s_t = cpool.tile([128, 1], F32)      # -V1S (relu bias)
            nc.vector.memset(sqsc_t[:], SQ_SCALE)
            nc.vector.memset(nv1s_t[:], -V1S)
            nc.gpsimd.iota(iotah[:], pattern=[[1, C]], base=0,
                           channel_multiplier=0,
                           allow_small_or_imprecise_dtypes=True)
            nc.vector.tensor_scalar(iotah[:], iotah[:], 1.5, None, OP.add)
            iob = iotah[:].unsqueeze(1).broadcast_to([128, L, C])

            for k in range(NT):
                xt = pool.tile([128, C * L], F32, tag="xt")
                kb = chpool.tile([128, C * L], F32, tag="kb")
                dA = pool.tile([128, C * L], F32, tag="dA")
                dB = pool.tile([128, C * L], F32, tag="dB")
                sA = pool.tile([128, C * L], F32, tag="sA")
                sB = pool.tile([128, C * L], F32, tag="sB")
                z = chpool.tile([128, C * L], F32, tag="z")
                keys = pool.tile([128, C * L], F32, tag="keys")
                cmblk = chpool.tile([128, L * 8], F32, tag="cmblk")

                nc.sync.dma_start(xt[:], x_d[:, k * C * L:(k + 1) * C * L])

                x3 = xt[:].rearrange("p (l c) -> p l c", c=C)
                kb3 = kb[:].rearrange("p (l c) -> p l c", c=C)
                dA3 = dA[:].rearrange("p (l c) -> p l c", c=C)
                dB3 = dB[:].rearrange("p (l c) -> p l c", c=C)
                sA3 = sA[:].rearrange("p (l c) -> p l c", c=C)
                sB3 = sB[:].rearrange("p (l c) -> p l c", c=C)
                z3 = z[:].rearrange("p (l c) -> p l c", c=C)
                k3 = keys[:].rearrange("p (l c) -> p l c", c=C)

                # kb = x/16 + (j + 1.5): key floor j+1 keeps the 2^23 round
                # trick in ulp==1 territory even for j=0 with x < -4
                nc.vector.scalar_tensor_tensor(kb3, x3, 1.0 / 16, iob,
                                               OP.mult, OP.add)

                for t in range(C):
                    r = t + 1
                    vcol = x3[:, :, t:t + 1].broadcast_to([128, L, r])
                    # d = x[0:r] - v (pristine x); widest steps on DVE to
                    # rebalance the saturated GPSIMD
                    d3 = dA3 if t % 2 == 0 else dB3
                    seng = nc.vector if t >= DVE_SUB_T else nc.gpsimd
                    seng.tensor_tensor(d3[:, :, 0:r], x3[:, :, 0:r],
                                       vcol, OP.subtract)
                    # ACT: s = (d*2^20)^2 = d^2 * 2^40 (exact); s ping-pongs
                    # between two tiles so ACT isn't chained to the DVE
                    # consumer of the previous step's s.
                    s3 = sA3 if t % 2 == 0 else sB3
                    nc.scalar.activation(s3[:, :, 0:r], d3[:, :, 0:r],
                                         AF.Square, scale=sqsc_t[:])
                    # DVE: z = (s - V1S) max kb: match -> kb, else >= 1024
                    nc.vector.scalar_tensor_tensor(z3[:, :, 0:r],
                                                   s3[:, :, 0:r], V1S,
                                                   kb3[:, :, 0:r],
                                                   OP.subtract, OP.max)
                    # DVE: keys[:,:,t] = min_j z
                    nc.vector.tensor_reduce(k3[:, :, t:t + 1], z3[:, :, 0:r],
                                            mybir.AxisListType.X, OP.min)
                    if t < C - 1:
                        # DVE: cm = (key < t+1)*1e30, written contiguous into
                        # the slot-major 8-slot ring (keeps the whole
                        # MIN->cm->punch chain on DVE, no cross-engine hop)
                        w = t % 8
                        nc.vector.tensor_scalar(
                            cmblk[:, w * L:(w + 1) * L],
                            k3[:, :, t:t + 1].squeeze(2),
                            float(t + 1), 1e30, OP.is_lt, OP.mult)
                        # DVE: windowed punch; out contiguous-inner, ring
                        # read strided-inner. Re-adding old punches only
                        # scales their 1e30.
                        B = (t // 8) * 8
                        cbs = cmblk[:].rearrange(
                            "p (c l) -> p c l", l=L).transpose([0, 2, 1])
                        nc.vector.tensor_tensor(
                            kb3[:, :, B:t + 1], kb3[:, :, B:t + 1],
                            cbs[:, :, 0:w + 1], OP.add)

                # unpack values: j1 = round(keys - 0.5) via 2^23 trick (exact,
                # ACT adds are fp32 RN), out = (keys - j1) * 16 - 8
                nc.scalar.activation(dA[:], keys[:], AF.Copy, bias=8388607.5)
                nc.scalar.activation(dA[:], dA[:], AF.Copy, bias=-8388608.0)
                nc.vector.tensor_tensor(keys[:], keys[:], dA[:], OP.subtract)
                nc.scalar.activation(keys[:], keys[:], AF.Copy,
                                     bias=-8.0, scale=16.0)
                nc.sync.dma_start(o_d[:, k * C * L:(k + 1) * C * L], keys[:])

    nc.compile()
    return nc


def _get_nc():
    if "nc" not in _CACHE:
        _CACHE["nc"] = _build()
    return _CACHE["nc"]


def kernel(x, _trace=False):
    assert x.shape == SHAPE and x.dtype == np.float32
    nc = _get_nc()
    lines = np.ascontiguousarray(x).reshape(NCORES, 128, LPP * C)
    in_maps = [{"x": lines[i]} for i in range(NCORES)]
    res = run_bass_kernel_spmd(nc, in_maps, list(range(NCORES)),
                               trace=_trace)
    outs = np.stack([res.results[i]["out"] for i in range(NCORES)])
    full = outs.reshape(SHAPE)
    if _trace:
        return full, res
    return full



# revision 2
# speedup vs baseline: 1.0581x; 1.0581x over previous
"""Trainium2 Bass kernel for nn_ClusteringLayer: per-cacheline serial
near-duplicate clustering (threshold 0.1, cacheline 64).

Algorithm (per line of 64 values, walked left to right): each value snaps
to the FIRST earlier base within |d| < 0.1, else becomes a new base.

Mapping: lines live on partitions (L=49 lines per partition per tile).
For step t, slots 0..t of each line are compared against value t. A packed
key kb_j = j + 1.5 + x_j/16 carries (slot, value) through a penalized
min-reduce, so the first matching base's index AND value come out of one
reduce with no gather. Non-base slots are "punched" by adding 1e30 to
their key. Match decisions are exact: d computed in fp32, Square(d*2^20)
= d^2*2^40 exactly (pow2 scale commutes with rounding), compared against
V1*2^40 where V1 = nextafter(round(0.1f^2), 0).

Schedule: GPSIMD does all subtracts (the only big op its ISA supports),
ACT squares IN PLACE (d tile doubles as s), DVE runs the serial chain
(combine STT -> segmented min-reduce -> cm -> width-1 punch). TWO tiles'
chains are interleaved at instruction level so every adjacent DVE
instruction is independent -> pipeline drain gaps are hidden. Width-1
punch (column t only) replaces the baseline's 8-wide windowed punch.

Sharding: pure data parallel over lines, 100352 lines per core x 8 cores.
"""
import numpy as np

import concourse.bacc as bacc
import concourse.mybir as mybir
import concourse.tile as tile
from concourse.bass_utils import run_bass_kernel_spmd

F32 = mybir.dt.float32
OP = mybir.AluOpType
AF = mybir.ActivationFunctionType

SHAPE = (64, 64, 112, 112)
C = 64                      # cacheline
NCORES = 8
TOTAL = 64 * 64 * 112 * 112     # 51380224
NLINES = TOTAL // C             # 802816
LPC = NLINES // NCORES          # 100352 lines per core
LPP = LPC // 128                # 784 lines per partition
L = 49                          # lines per partition per tile
NT = LPP // L                   # tiles per core
P = 2                           # interleaved tiles
ND = 2                          # d-tile rotation depth

# exact threshold: match <=> d^2 <= V1 <=> (d*2^20)^2 <= V1*2^40
V2 = np.float32(np.float32(0.1) * np.float32(0.1))
V1 = np.nextafter(V2, np.float32(0), dtype=np.float32)
SQ_SCALE = float(np.float32(2.0 ** 20))
V1S = float(np.float32(V1.astype(np.float64) * 2.0 ** 40))

_CACHE = {}


def _build(reps=1, gpsub=1, nd=ND):
    nc = bacc.Bacc("TRN2", target_bir_lowering=False, debug=False)
    x_d = nc.dram_tensor("x", [128, LPP * C], F32, kind="ExternalInput")
    o_d = nc.dram_tensor("out", [128, LPP * C], F32, kind="ExternalOutput")

    with tile.TileContext(nc) as tc:
        with (
            tc.tile_pool(name="const", bufs=1) as cpool,
            tc.tile_pool(name="work", bufs=1) as pool,
            tc.tile_pool(name="chain", bufs=1) as chpool,
        ):
            iotah = cpool.tile([128, C], F32)
            sqsc_t = cpool.tile([128, 1], F32)
            nc.vector.memset(sqsc_t[:], SQ_SCALE)
            nc.gpsimd.iota(iotah[:], pattern=[[1, C]], base=0,
                           channel_multiplier=0,
                           allow_small_or_imprecise_dtypes=True)
            nc.vector.tensor_scalar(iotah[:], iotah[:], 1.5, None, OP.add)
            iob = iotah[:].unsqueeze(1).broadcast_to([128, L, C])

            for rep in range(reps):
                for kp in range(NT // P):
                    slots = []
                    for s in range(P):
                        k = kp * P + s
                        xt = pool.tile([128, C * L], F32, tag=f"xt{s}",
                                       name=f"xt{s}", bufs=2)
                        ds = [pool.tile([128, C * L], F32, tag=f"d{s}_{i}",
                                        name=f"d{s}_{i}")
                              for i in range(nd)]
                        kb = chpool.tile([128, C * L], F32, tag=f"kb{s}",
                                         name=f"kb{s}")
                        z = chpool.tile([128, C * L], F32, tag=f"z{s}",
                                        name=f"z{s}")
                        cm = chpool.tile([128, L], F32, tag=f"cm{s}",
                                         name=f"cm{s}")
                        keys = pool.tile([128, C * L], F32, tag=f"keys{s}",
                                         name=f"keys{s}")
                        nc.sync.dma_start(
                            xt[:], x_d[:, k * C * L:(k + 1) * C * L])
                        slots.append(dict(
                            k=k, xt=xt, ds=ds,
                            x3=xt[:].rearrange("p (l c) -> p l c", c=C),
                            d3s=[d[:].rearrange("p (l c) -> p l c", c=C)
                                 for d in ds],
                            kb3=kb[:].rearrange("p (l c) -> p l c", c=C),
                            z3=z[:].rearrange("p (l c) -> p l c", c=C),
                            cm=cm,
                            cm3=cm[:].rearrange("p (l c) -> p l c", c=1),
                            keys=keys,
                            k3=keys[:].rearrange("p (l c) -> p l c", c=C),
                        ))

                    for sl in slots:
                        nc.vector.scalar_tensor_tensor(
                            sl["kb3"], sl["x3"], 1.0 / 16, iob,
                            OP.mult, OP.add)

                    for t in range(C):
                        r = t + 1
                        use_gp = gpsub > 0 and (t % gpsub == gpsub - 1)
                        seng = nc.gpsimd if use_gp else nc.vector
                        for sl in slots:
                            vcol = sl["x3"][:, :, t:t + 1].broadcast_to(
                                [128, L, r])
                            d3 = sl["d3s"][t % nd]
                            seng.tensor_tensor(d3[:, :, 0:r],
                                               sl["x3"][:, :, 0:r],
                                               vcol, OP.subtract)
                        for sl in slots:
                            d3 = sl["d3s"][t % nd]
                            nc.scalar.activation(d3[:, :, 0:r],
                                                 d3[:, :, 0:r],
                                                 AF.Square, scale=sqsc_t[:])
                        for sl in slots:
                            d3 = sl["d3s"][t % nd]
                            nc.vector.scalar_tensor_tensor(
                                sl["z3"][:, :, 0:r], d3[:, :, 0:r], V1S,
                                sl["kb3"][:, :, 0:r], OP.subtract, OP.max)
                        for sl in slots:
                            nc.vector.tensor_reduce(
                                sl["k3"][:, :, t:t + 1],
                                sl["z3"][:, :, 0:r],
                                mybir.AxisListType.X, OP.min)
                        if t < C - 1:
                            for sl in slots:
                                nc.vector.tensor_scalar(
                                    sl["cm"][:, :],
                                    sl["k3"][:, :, t:t + 1].squeeze(2),
                                    float(t + 1), 1e30, OP.is_lt, OP.mult)
                            for sl in slots:
                                nc.vector.tensor_tensor(
                                    sl["kb3"][:, :, t:t + 1],
                                    sl["kb3"][:, :, t:t + 1],
                                    sl["cm3"][:, :, 0:1], OP.add)

                    # unpack: j1 = round(keys - 0.5) via 2^23 trick (exact),
                    # out = (keys - j1) * 16 - 8
                    for sl in slots:
                        d0 = sl["ds"][0]
                        keys = sl["keys"]
                        nc.scalar.activation(d0[:], keys[:], AF.Copy,
                                             bias=8388607.5)
                        nc.scalar.activation(d0[:], d0[:], AF.Copy,
                                             bias=-8388608.0)
                        nc.vector.tensor_tensor(keys[:], keys[:], d0[:],
                                                OP.subtract)
                        nc.scalar.activation(keys[:], keys[:], AF.Copy,
                                             bias=-8.0, scale=16.0)
                        nc.sync.dma_start(
                            o_d[:, sl["k"] * C * L:(sl["k"] + 1) * C * L],
                            keys[:])

    nc.compile()
    return nc


def build(reps=1, gpsub=1, nd=ND):
    return _build(reps=reps, gpsub=gpsub, nd=nd)


def _get_nc():
    if "nc" not in _CACHE:
        _CACHE["nc"] = _build()
    return _CACHE["nc"]


def kernel(x):
    assert x.shape == SHAPE and x.dtype == np.float32
    nc = _get_nc()
    lines = np.ascontiguousarray(x).reshape(NCORES, 128, LPP * C)
    in_maps = [{"x": lines[i]} for i in range(NCORES)]
    res = run_bass_kernel_spmd(nc, in_maps, list(range(NCORES)))
    outs = np.stack([res.results[i]["out"] for i in range(NCORES)])
    return outs.reshape(SHAPE)
